# revision 8
# baseline (speedup 1.0000x reference)
"""CFAR box-filter kernel (31x31 / 11x11 box sums + ratio) for Trainium2.

Data-parallel over batch: 32 images -> 8 NeuronCores, 4 images each.
Per 128-row chunk (software-pipelined, 4-deep skew):
  - h11/h31 horizontal window sums via DVE sliding-window scans
    (state = state + x[j] - x[j-K], fp32 state, fp16 in/out),
  - vertical band sums as fp16 matmuls into PSUM (weights carry 1/121
    and +-1/840; halo rows from neighbor chunks' h tiles gathered by
    SBUF-SBUF DMA, one combined halo matmul per psum target),
  - ACT copies front out (fp16) and computes 1/back via the Reciprocal
    activation table, POOL multiplies front * (1/back).
I/O is fp16 end to end; the host pads/converts in, upcasts out.
"""

import os
import sys

import numpy as np

for _p in ("/opt/trn_rl_repo", "/root/.axon_site/_ro/trn_rl_repo"):
    if os.path.isdir(_p) and _p not in sys.path:
        sys.path.insert(0, _p)
        break

import concourse.bass as bass
import concourse.tile as tile
from concourse import bacc
from concourse import mybir
from concourse._compat import with_exitstack
from concourse.bass_utils import run_bass_kernel_spmd

B, H, W = 32, 1024, 1024
NCORES = 8
BPC = B // NCORES            # images per core
T = H // 128                 # row chunks per image
PADL, PADR = 32, 16
XW = PADL + W + PADR         # 1072
F32 = mybir.dt.float32
F16 = mybir.dt.float16

R_IN, R_OUT = 5, 15
AREA_FRONT = 121.0
AREA_BACK = 840.0

# sliding-scan output geometry: h11[k] = 11-window ending at padded col 11+k,
# so image col c lives at k = c + OFF11 (similarly for h31)
OFF11, L11 = PADL + R_IN - 11, PADL + R_IN + (W - 1) - 11 + 1    # 26, 1050
OFF31, L31 = PADL + R_OUT - 31, PADL + R_OUT + (W - 1) - 31 + 1  # 16, 1040


def _weights() -> dict[str, np.ndarray]:
    m = np.arange(128)[None, :]

    def blk(radius, val):
        k = np.arange(128)[:, None]
        return ((np.abs(k - m) <= radius) * val).astype(np.float16)

    def halo(rows):
        # rows: list of (delta, radius, val); delta = image-row offset of the
        # gathered halo row relative to this chunk's first output row
        w = np.zeros((len(rows), 128), np.float32)
        for i, (delta, radius, val) in enumerate(rows):
            w[i] = (np.abs(delta - m[0]) <= radius) * val
        return w.astype(np.float16)

    prev31 = [(-R_OUT + i, R_OUT, 1 / AREA_BACK) for i in range(15)]
    next31 = [(128 + i, R_OUT, 1 / AREA_BACK) for i in range(15)]
    prev11 = [(-R_IN + i, R_IN, -1 / AREA_BACK) for i in range(5)]
    next11 = [(128 + i, R_IN, -1 / AREA_BACK) for i in range(5)]
    prev11f = [(-R_IN + i, R_IN, 1 / AREA_FRONT) for i in range(5)]
    next11f = [(128 + i, R_IN, 1 / AREA_FRONT) for i in range(5)]

    return {
        "wf_blk": blk(R_IN, 1 / AREA_FRONT),
        "wb_blk": blk(R_OUT, 1 / AREA_BACK),
        "wn_blk": blk(R_IN, -1 / AREA_BACK),
        # h11-halo rows first so both halo matmuls read base partition 0
        "whI_b": halo(prev11 + next11 + prev31 + next31),   # [40,128]
        "whI_f": halo(prev11f + next11f),                   # [10,128]
        "whN_b": halo(next11 + next31),                     # [20,128] t=0
        "whN_f": halo(next11f),                             # [5,128]
        "whP_b": halo(prev11 + prev31),                     # [20,128] t=7
        "whP_f": halo(prev11f),                             # [5,128]
    }


def _act_recip(nc, out_ap, in_ap):
    """InstActivation(func=Reciprocal): measured ~1e-5 max rel err on HW.

    bass's wrapper refuses Reciprocal; emit it directly (Copy and
    Reciprocal share the reciprocal_and_small table set, so no per-op
    table reloads)."""
    se = nc.scalar
    imm = lambda v: mybir.ImmediateValue(dtype=mybir.dt.float32, value=v)
    return se.add_instruction(
        mybir.InstActivation(
            name=nc.get_next_instruction_name(),
            func=mybir.ActivationFunctionType.Reciprocal,
            ins=[se.lower_ap(in_ap), imm(0.0), imm(1.0), imm(0.0)],
            outs=[se.lower_ap(out_ap)],
        )
    )


@with_exitstack
def _cfar_tile_kernel(ctx, tc, x_d, o_d, w_d, n_img, reps=1):
    nc = tc.nc
    ADD = mybir.AluOpType.add
    SUB = mybir.AluOpType.subtract
    MUL = mybir.AluOpType.mult

    const = ctx.enter_context(tc.tile_pool(name="const", bufs=1))
    wt = {}
    for name, dram_ap in w_d.items():
        t = const.tile(list(dram_ap.shape), F16, tag=name)
        nc.sync.dma_start(t[:], dram_ap)
        wt[name] = t

    xp = ctx.enter_context(tc.tile_pool(name="xp", bufs=3))
    h11p = ctx.enter_context(tc.tile_pool(name="h11p", bufs=5))
    h31p = ctx.enter_context(tc.tile_pool(name="h31p", bufs=5))
    gp = ctx.enter_context(tc.tile_pool(name="gp", bufs=3))
    pp = ctx.enter_context(tc.tile_pool(name="pp", bufs=2, space="PSUM"))
    o1p = ctx.enter_context(tc.tile_pool(name="o1p", bufs=3))
    rp = ctx.enter_context(tc.tile_pool(name="rp", bufs=3))
    o0p = ctx.enter_context(tc.tile_pool(name="o0p", bufs=3))

    NSTEP = n_img * T

    def one_pass():
        xt: dict[int, object] = {}
        h11: dict[int, object] = {}
        h31: dict[int, object] = {}
        gt: dict[int, object] = {}
        psf: dict[int, object] = {}
        psb: dict[int, object] = {}
        o1: dict[int, object] = {}
        rr: dict[int, object] = {}

        def load(j):
            img, t = divmod(j, T)
            xt[j] = xp.tile([128, XW], F16, tag="xt", name=f"xt{j}")
            nc.sync.dma_start(xt[j][:], x_d[img, 128 * t : 128 * (t + 1), :])

        def scans(j):
            h11[j] = h11p.tile([128, L11], F16, tag="h11", name=f"h11_{j}")
            nc.vector.tensor_tensor_scan(
                h11[j][:], xt[j][:, 11 : 11 + L11], xt[j][:, 0:L11], 0.0, ADD, SUB
            )
            h31[j] = h31p.tile([128, L31], F16, tag="h31", name=f"h31_{j}")
            nc.vector.tensor_tensor_scan(
                h31[j][:], xt[j][:, 31 : 31 + L31], xt[j][:, 0:L31], 0.0, ADD, SUB
            )
            xt.pop(j)

        def gather(j):
            # halo rows for chunk j's vertical windows, copied out of the
            # neighbor chunks' h tiles with column offsets pre-aligned
            t = j % T
            g = gp.tile([40, W], F16, tag="g", name=f"g{j}")
            gt[j] = g
            D = nc.gpsimd.dma_start
            if t == 0:
                D(g[0:5, :], h11[j + 1][0:5, OFF11 : OFF11 + W])
                D(g[5:20, :], h31[j + 1][0:15, OFF31 : OFF31 + W])
            elif t == T - 1:
                D(g[0:5, :], h11[j - 1][123:128, OFF11 : OFF11 + W])
                D(g[5:20, :], h31[j - 1][113:128, OFF31 : OFF31 + W])
            else:
                D(g[0:5, :], h11[j - 1][123:128, OFF11 : OFF11 + W])
                D(g[5:10, :], h11[j + 1][0:5, OFF11 : OFF11 + W])
                D(g[10:25, :], h31[j - 1][113:128, OFF31 : OFF31 + W])
                D(g[25:40, :], h31[j + 1][0:15, OFF31 : OFF31 + W])

        def matmuls(j):
            t = j % T
            if t == 0:
                wb, wf, fsl, G = wt["whN_b"], wt["whN_f"], (0, 5), 20
            elif t == T - 1:
                wb, wf, fsl, G = wt["whP_b"], wt["whP_f"], (0, 5), 20
            else:
                wb, wf, fsl, G = wt["whI_b"], wt["whI_f"], (0, 10), 40
            g = gt.pop(j)
            psf[j] = pp.tile([128, W], F32, tag="psf", name=f"psf{j}")
            psb[j] = pp.tile([128, W], F32, tag="psb", name=f"psb{j}")
            MM = nc.tensor.matmul
            for s0 in (0, 512):
                s = slice(s0, s0 + 512)
                MM(psf[j][:, s], wt["wf_blk"][:],
                   h11[j][:, OFF11 + s0 : OFF11 + s0 + 512], start=True, stop=False)
                MM(psf[j][:, s], wf[:], g[fsl[0] : fsl[1], s], start=False, stop=True)
                MM(psb[j][:, s], wt["wb_blk"][:],
                   h31[j][:, OFF31 + s0 : OFF31 + s0 + 512], start=True, stop=False)
                MM(psb[j][:, s], wt["wn_blk"][:],
                   h11[j][:, OFF11 + s0 : OFF11 + s0 + 512], start=False, stop=False)
                MM(psb[j][:, s], wb[:], g[0:G, s], start=False, stop=True)

        def act(j):
            o1[j] = o1p.tile([128, W], F16, tag="o1", name=f"o1_{j}")
            nc.scalar.copy(o1[j][:], psf.pop(j)[:])
            rr[j] = rp.tile([128, W], F16, tag="r", name=f"r{j}")
            _act_recip(nc, rr[j][:], psb.pop(j)[:])

        def poolout(j):
            img, t = divmod(j, T)
            o0 = o0p.tile([128, W], F16, tag="o0", name=f"o0_{j}")
            nc.gpsimd.tensor_tensor(o0[:], o1[j][:], rr.pop(j)[:], MUL)
            rows = slice(128 * t, 128 * (t + 1))
            nc.gpsimd.dma_start(o_d[img, rows, :], o0[:])
            nc.gpsimd.dma_start(o_d[n_img + img, rows, :], o1.pop(j)[:])
            # drop old h tiles no longer needed (t+1 neighbors read them)
            for d in (h11, h31):
                for k in [k for k in d if k < j]:
                    d.pop(k)

        load(0)
        for i in range(NSTEP + 4):
            if i >= 4 and i - 4 < NSTEP:
                poolout(i - 4)
            if i >= 3 and i - 3 < NSTEP:
                act(i - 3)
            if i >= 2 and i - 2 < NSTEP:
                matmuls(i - 2)
            if i < NSTEP:
                scans(i)
            if i >= 1 and i - 1 < NSTEP:
                gather(i - 1)
            if i + 1 < NSTEP:
                load(i + 1)

    if reps == 1:
        one_pass()
    else:
        with tc.For_i(0, reps, 1):
            one_pass()


def build(n_img: int = BPC, reps: int = 1):
    nc = bacc.Bacc("TRN2", target_bir_lowering=False, debug=False)
    x_d = nc.dram_tensor("x", [n_img, H, XW], F16, kind="ExternalInput").ap()
    o_d = nc.dram_tensor("out", [2 * n_img, H, W], F16, kind="ExternalOutput").ap()
    wts = _weights()
    w_d = {
        k: nc.dram_tensor(k, list(v.shape), F16, kind="ExternalInput").ap()
        for k, v in wts.items()
    }
    with tile.TileContext(nc) as tc:
        _cfar_tile_kernel(tc, x_d, o_d, w_d, n_img, reps)
    nc.compile()
    return nc, wts


_CACHE: dict = {}


def _prep_input(x: np.ndarray) -> np.ndarray:
    xs = np.zeros((B, H, XW), dtype=np.float16)
    xs[:, :, PADL : PADL + W] = x[:, 0]
    return xs


def kernel(x: np.ndarray) -> np.ndarray:
    x = np.asarray(x, dtype=np.float32)
    assert x.shape == (B, 1, H, W), x.shape
    if "nc" not in _CACHE:
        _CACHE["nc"], _CACHE["wts"] = build(BPC)
    nc, wts = _CACHE["nc"], _CACHE["wts"]
    xs = _prep_input(x)
    in_maps = []
    for i in range(NCORES):
        m = {"x": np.ascontiguousarray(xs[BPC * i : BPC * (i + 1)])}
        m.update(wts)
        in_maps.append(m)
    res = run_bass_kernel_spmd(nc, in_maps, list(range(NCORES))).results
    out = np.empty((2 * B, 1, H, W), dtype=np.float32)
    for i in range(NCORES):
        o = res[i]["out"].astype(np.float32)
        out[BPC * i : BPC * (i + 1), 0] = o[:BPC]
        out[B + BPC * i : B + BPC * (i + 1), 0] = o[BPC:]
    return out
